# revision 48
# baseline (speedup 1.0000x reference)
"""Trainium2 Bass kernel for channel-wise ("transposed") attention.

Reference computation (per batch b, X = x_in[b] reshaped [N=16384, C=256]):
    Q = X Wq ; K = X Wk ; V = X Wv            (columns l2-normalized over tokens for Q,K)
    attn[h,i,j] = softmax_j( khat_i . qhat_j * rescale[h] )   (32x32 per head)
    out = (A_bd @ V^T)^T Wp + bp

Algebraic reduction (validated vs reference):
    S    = X^T X                      [256,256]   (only pass-1 reduction needed)
    P1   = S Wq ; P2 = S Wk
    G    = Wk^T P1                    (raw cross-gram K^T Q)
    nq2  = diag(Wq^T P1) ; nk2 = diag(Wk^T P2)
    L    = G * rk[i] * (rq*rescale_expanded)[j] ;  A = blockdiag-softmax_j(exp(L))
    Wbig = Wv @ (A_bd^T Wp)           [256,256]
    out  = X @ Wbig + bp

Schedule (per core = one batch, data parallel, no collectives):
  pass 1   stream X f32 -> bf16 SBUF (casting DMA halves modeled DMA cost);
           X stays fully resident (64KB/partition).  PE does only the S
           accumulation here, so pass 1 is input-DMA + S-matmul bound.
  phase B  tiny 256x256 chains -> Wbig.  Single activation-table set
           (ln/exp/copy) loaded once at t=0: zero on-path table loads;
           rsqrt via exp(-0.5 ln x); rescale pre-folded into a scaled Wq
           copy used only by the norm fork (exact for rescale > 0).
  pass 2   per output group: transpose that group's X tiles on PE (bf16,
           1 cyc/row) -> evict to xT -> out = X @ Wbig + bp -> f32 DMA out.
           Output DMA (46.6us) is the bound; transposes live in PE slack.
"""

import sys

if "/opt/trn_rl_repo" not in sys.path:
    sys.path.insert(0, "/opt/trn_rl_repo")

from contextlib import ExitStack

import numpy as np

import concourse.bass as bass
import concourse.tile as tile
from concourse import bacc, mybir
from concourse import bass_utils
from concourse.bass import ds, ts
from concourse.bass_interp import get_hw_module
from concourse.masks import make_identity

F32 = mybir.dt.float32
F32R = mybir.dt.float32r
BF16 = mybir.dt.bfloat16
ALU = mybir.AluOpType
ACTF = mybir.ActivationFunctionType
PSUM = bass.MemorySpace.PSUM

N_CORES = 8
B, H, W, C = 8, 128, 128, 256
HEADS, DH = 8, 32
N = H * W            # 16384 tokens per batch
P = 128              # partitions / token tile
NT = N // P          # 128 token tiles
DMA_TILES = 8        # token tiles per DMA (1 MiB f32 chunks)
NG = NT // DMA_TILES # 16 groups
NCHUNK = C // P      # 2 channel chunks

# act_func_sets index of natural_log_exp_and_others: {ln, exp, copy, ...}
ACT_SET_LN_EXP = 6


def _build_kernel(nc: bacc.Bacc):
    x_dram = nc.dram_tensor("x_in", [N, C], F32, kind="ExternalInput").ap()
    wq_dram = nc.dram_tensor("Wq", [C, C], F32, kind="ExternalInput").ap()
    wk_dram = nc.dram_tensor("Wk", [C, C], F32, kind="ExternalInput").ap()
    wv_dram = nc.dram_tensor("Wv", [C, C], F32, kind="ExternalInput").ap()
    resc_dram = nc.dram_tensor("rescale", [HEADS, 1, 1], F32, kind="ExternalInput").ap()
    wp_dram = nc.dram_tensor("Wp", [C, C], F32, kind="ExternalInput").ap()
    bp_dram = nc.dram_tensor("bp", [C], F32, kind="ExternalInput").ap()
    out_dram = nc.dram_tensor("out", [N, C], F32, kind="ExternalOutput").ap()

    with tile.TileContext(nc) as tc, ExitStack() as top:
        consts = top.enter_context(tc.tile_pool(name="consts", bufs=1))
        xt_pool = top.enter_context(tc.tile_pool(name="xt", bufs=1))
        xf_pool = top.enter_context(tc.tile_pool(name="xfull", bufs=1))
        s_stack = ExitStack()
        s_pool = s_stack.enter_context(tc.tile_pool(name="spsum", bufs=1, space=PSUM))

        # ------------- const tiles (instructions emitted inside pass-1 g==0) -------------
        identity_f = consts.tile([P, P], F32)
        identity = consts.tile([P, P], BF16)     # bf16: 1 cyc/row transposes
        p8 = consts.tile([HEADS, C], F32)        # p8[h,c] = 1 iff c//32 == h
        p8_r = consts.tile([HEADS, C], F32R)
        bdmask = consts.tile([P, NCHUNK, C], F32)  # block-diag head mask chunks
        ones_col_f = consts.tile([P, 1], F32)
        ones_col = consts.tile([P, 1], F32R)     # [128,1] ones: column-sum matmuls
        ones_row = consts.tile([1, P], F32)      # [1,128] ones: partition broadcast
        ones_row_r = consts.tile([1, P], F32R)

        # weight tiles (DMAs issued after the x loads to keep x at queue head)
        wqk = consts.tile([P, NCHUNK, 2 * C], F32)       # [Wq | Wk] row chunks
        wp_sb = consts.tile([P, NCHUNK, C], F32)
        wv_sb = consts.tile([P, NCHUNK, C], F32)
        wvT = consts.tile([P, NCHUNK, C], F32R)          # wvT[p,k,c] = Wv[c, 128k+p]
        wqk_r = consts.tile([P, NCHUNK, 2 * C], F32R)    # rounded copies for f32r mms
        wp_r = consts.tile([P, NCHUNK, C], F32R)
        bp_sb = consts.tile([1, C], F32)
        bp2_r = consts.tile([1, 2 * C], F32R)    # [bp | bp] row for bias matmuls
        resc_p = consts.tile([HEADS, 1], F32)
        resc_r = consts.tile([HEADS, 1], F32R)
        rexp_row = consts.tile([1, C], F32)      # rescale broadcast over head blocks
        rexp1i = consts.tile([1, C], F32)        # rexp^-1 row
        rexp2i = consts.tile([1, C], F32)        # rexp^-2 row
        wq_scaled = consts.tile([P, NCHUNK, C], F32)  # Wq * rexp^-2 (qp/nq2 only)
        bias_bc = consts.tile([P, 2 * C], F32)   # [bp | bp] broadcast down partitions
        wbig0 = consts.tile([P, C], BF16)
        wbig1 = consts.tile([P, C], BF16)
        wbig_l = [wbig0, wbig1]

        # X^T (bf16), built in pass 2; one tensor per output group so the
        # out-matmuls of group g depend only on group g's transposes
        xg = [xf_pool.tile([P, DMA_TILES, C], BF16, name=f"xg{g}") for g in range(NG)]
        GROUPS = [2, 2, 2, 2] + [8] * 15
        assert sum(GROUPS) == NT
        starts = [sum(GROUPS[:i]) for i in range(len(GROUPS))]
        xTg = [
            xt_pool.tile([P, NCHUNK, gsz * P], BF16, name=f"xTg{gi}")
            for gi, gsz in enumerate(GROUPS)
        ]

        s_ps0 = s_pool.tile([P, C], F32, space=PSUM)
        s_ps1 = s_pool.tile([P, C], F32, space=PSUM)
        s_ps = [s_ps0, s_ps1]

        # ---------------- pass 1: load X (bf16), S = X^T X ----------------
        with tc.tile_pool(name="tp", bufs=4, space=PSUM) as tp_pool:
            for g in range(NG):
                if g == 0:
                    # small first piece so PE starts sooner
                    for lo, n_t in ((0, 4), (4, 4)):
                        nc.gpsimd.dma_start(
                            xg[g][:, ds(lo, n_t), :],
                            x_dram[ds((g * DMA_TILES + lo) * P, n_t * P), :].rearrange(
                                "(a p) c -> p a c", p=P
                            ),
                        )
                else:
                    nc.gpsimd.dma_start(
                        xg[g][:],
                        x_dram[ds(g * DMA_TILES * P, DMA_TILES * P), :].rearrange(
                            "(a p) c -> p a c", p=P
                        ),
                    )
                if g == 0:
                    # single activation-table load for the whole kernel
                    nc.scalar.add_instruction(
                        mybir.InstLoadActFuncSet(
                            name=nc.get_next_instruction_name(),
                            act_func_set_id=ACT_SET_LN_EXP,
                            ins=[],
                            outs=[],
                        )
                    )
                    make_identity(nc, identity_f[:])
                    nc.vector.tensor_copy(identity[:], identity_f[:])
                    nc.gpsimd.memset(p8[:], 0.0)
                    nc.gpsimd.affine_select(
                        out=p8[:].rearrange("p (b i) -> p b i", i=DH),
                        in_=p8[:].rearrange("p (b i) -> p b i", i=DH),
                        compare_op=ALU.not_equal,
                        fill=1.0,
                        base=0,
                        pattern=[[-1, HEADS], [0, DH]],
                        channel_multiplier=1,
                    )
                    nc.vector.tensor_copy(p8_r[:], p8[:])
                    nc.gpsimd.memset(bdmask[:], 0.0)
                    for r in range(NCHUNK):
                        for a2 in range(P // DH):
                            nc.gpsimd.memset(
                                bdmask[ts(a2, DH), r, ds(r * P + a2 * DH, DH)], 1.0
                            )
                    nc.gpsimd.memset(ones_col_f[:], 1.0)
                    nc.vector.tensor_copy(ones_col[:], ones_col_f[:])
                    nc.gpsimd.memset(ones_row[:], 1.0)
                    nc.vector.tensor_copy(ones_row_r[:], ones_row[:])
                if g == 1:
                    # weight/bias loads + prep: issued behind the first x chunk
                    for k in range(NCHUNK):
                        nc.sync.dma_start(wqk[:, k, 0:C], wq_dram[ts(k, P), :])
                        nc.sync.dma_start(wqk[:, k, C : 2 * C], wk_dram[ts(k, P), :])
                        nc.sync.dma_start(wp_sb[:, k, :], wp_dram[ts(k, P), :])
                        nc.sync.dma_start(wv_sb[:, k, :], wv_dram[ts(k, P), :])
                    nc.sync.dma_start(bp_sb[:], bp_dram.rearrange("(a c) -> a c", a=1))
                    nc.sync.dma_start(resc_p[:], resc_dram.rearrange("h a b -> h (a b)"))
                    for k in range(NCHUNK):
                        nc.vector.tensor_copy(wqk_r[:, k, :], wqk[:, k, :])
                        nc.vector.tensor_copy(wp_r[:, k, :], wp_sb[:, k, :])
                    nc.vector.tensor_copy(bp2_r[:, 0:C], bp_sb[:])
                    nc.vector.tensor_copy(bp2_r[:, C : 2 * C], bp_sb[:])
                    nc.vector.tensor_copy(resc_r[:], resc_p[:])
            # S accumulation.  The weight-prep matmuls are interleaved right
            # where PE would otherwise stall waiting for early DMA groups, so
            # PE ramps once and never resets pstate.
            def s_tile(t, first=False, last=False):
                g, a = divmod(t, DMA_TILES)
                x_t = xg[g][:, a, :]
                for k in range(NCHUNK):
                    nc.tensor.matmul(
                        s_ps[k][:],
                        x_t[:, ts(k, P)],
                        x_t[:],
                        start=first and k == 0,
                        stop=last and k == 1,
                    )

            s_tile(0, first=True)
            s_tile(1)
            s_tile(2)
            s_tile(3)
            # prep block 1: Wv transposes, rescale row, bias broadcast (PE)
            for k in range(NCHUNK):
                for m in range(NCHUNK):
                    tpv = tp_pool.tile([P, P], F32, space=PSUM, tag="tp")
                    nc.tensor.transpose(
                        tpv[:].bitcast(F32), wv_sb[:, m, ts(k, P)], identity_f[:]
                    )
                    nc.vector.tensor_copy(wvT[:, k, ts(m, P)], tpv[:].bitcast(F32))
            rexp_ps = tp_pool.tile([P, C], F32, space=PSUM, tag="tp")
            nc.tensor.matmul(
                rexp_ps[0:1, :], resc_r[:], p8_r[:], start=True, stop=True
            )
            nc.vector.tensor_copy(rexp_row[:], rexp_ps[0:1, :])
            nc.vector.reciprocal(rexp1i[:], rexp_row[:])
            nc.vector.tensor_mul(rexp2i[:], rexp1i[:], rexp1i[:])
            bb_ps = tp_pool.tile([P, 2 * C], F32, space=PSUM, tag="tp")
            nc.tensor.matmul(
                bb_ps[:], ones_row_r[:], bp2_r[:], start=True, stop=True
            )
            nc.scalar.copy(bias_bc[:], bb_ps[:])
            for t in range(4, 8):
                s_tile(t)
            for t in range(8, 16):
                s_tile(t)
            # prep block 2: rexp^-2 broadcast + scaled Wq (norm-fork input)
            r2bc_ps = tp_pool.tile([P, C], F32, space=PSUM, tag="tp")
            nc.tensor.matmul(
                r2bc_ps[:], ones_row[:], rexp2i[:], start=True, stop=True
            )
            for k in range(NCHUNK):
                nc.vector.tensor_mul(wq_scaled[:, k, :], wqk[:, k, 0:C], r2bc_ps[:])
            for t in range(16, NT):
                s_tile(t, last=(t == NT - 1))

        # ---------------- phase B: 256x256 attention math ----------------
        # Per-chunk tensors so chunk-0 consumers never wait on chunk-1 writes.
        # PSUM evictions alternate DVE/ACT; the softmax path (P1 -> G -> t ->
        # e -> A -> T1 -> Wbig) is kept separate from the norm forks, which
        # read PSUM directly and merge only at the Exp.
        with tc.tile_pool(name="bwork", bufs=4, space=PSUM) as bwork, tc.tile_pool(
            name="bsmall", bufs=2, space=PSUM
        ) as bsmall, tc.tile_pool(name="bsb", bufs=1) as bsb:
            s_sbl = []
            for k in range(NCHUNK):
                s_k = bsb.tile([P, C], F32R, name=f"s_sb{k}", tag="ssb", bufs=2)
                if k == 0:
                    nc.vector.tensor_copy(s_k[:], s_ps[k][:])
                else:
                    nc.scalar.copy(s_k[:], s_ps[k][:])
                s_sbl.append(s_k)

            # P1 = S @ Wq, P2 = S @ Wk   (uses S symmetric: lhsT = S chunks)
            p1_psl, p2_psl = [], []
            for m in range(NCHUNK):
                pp = bwork.tile([P, C], F32, space=PSUM, name=f"p1ps{m}", tag="bw", bufs=4)
                for k in range(NCHUNK):
                    nc.tensor.matmul(
                        pp[:],
                        s_sbl[k][:, ts(m, P)],
                        wqk_r[:, k, 0:C],
                        start=(k == 0),
                        stop=(k == 1),
                    )
                p1_psl.append(pp)
            for m in range(NCHUNK):
                pp = bwork.tile([P, C], F32, space=PSUM, name=f"p2ps{m}", tag="bw", bufs=4)
                for k in range(NCHUNK):
                    nc.tensor.matmul(
                        pp[:],
                        s_sbl[k][:, ts(m, P)],
                        wqk_r[:, k, C : 2 * C],
                        start=(k == 0),
                        stop=(k == 1),
                    )
                p2_psl.append(pp)
            p1_sbl = []
            for m in range(NCHUNK):
                psb = bsb.tile([P, C], F32R, name=f"p1sb{m}", tag="p1sb", bufs=2)
                if m == 0:
                    nc.vector.tensor_copy(psb[:], p1_psl[m][:])
                else:
                    nc.scalar.copy(psb[:], p1_psl[m][:])
                p1_sbl.append(psb)

            # norm fork #1: nq2*rexp^-2 via wq_scaled; rq = rsqrt -> rq*rescale
            qpl = []
            for m in range(NCHUNK):
                qp = bsb.tile([P, C], F32R, name=f"qp{m}", tag="qp", bufs=2)
                nc.vector.tensor_mul(qp[:], wq_scaled[:, m, :], p1_psl[m][:])
                qpl.append(qp)
            nq2_ps = bsmall.tile([1, C], F32, space=PSUM, tag="bs")
            for k in range(NCHUNK):
                nc.tensor.matmul(
                    nq2_ps[:], ones_col[:], qpl[k][:], start=(k == 0), stop=(k == 1)
                )
            lnq = bsb.tile([1, C], F32)
            nc.scalar.activation(lnq[:], nq2_ps[:], ACTF.Ln)
            rq = bsb.tile([1, C], F32R)
            nc.scalar.activation(rq[:], lnq[:], ACTF.Exp, scale=-0.5)
            csbc_ps = bsmall.tile([P, C], F32, space=PSUM, tag="bs")
            nc.tensor.matmul(csbc_ps[:], ones_row_r[:], rq[:])
            csbc_sb = bsb.tile([P, C], F32)
            nc.scalar.copy(csbc_sb[:], csbc_ps[:])

            # softmax path: G = Wk^T P1
            g_psl = []
            for m in range(NCHUNK):
                gg = bwork.tile([P, C], F32, space=PSUM, name=f"gps{m}", tag="bw", bufs=4)
                for k in range(NCHUNK):
                    nc.tensor.matmul(
                        gg[:],
                        wqk_r[:, k, ds(C + m * P, P)],
                        p1_sbl[k][:],
                        start=(k == 0),
                        stop=(k == 1),
                    )
                g_psl.append(gg)

            # norm fork #2: Kgram = Wk^T P2, nk2 = diag, rk = nk2^-1/2
            p2_sbl = []
            for m in range(NCHUNK):
                psb = bsb.tile([P, C], F32R, name=f"p2sb{m}", tag="p2sb", bufs=2)
                if m == 0:
                    nc.vector.tensor_copy(psb[:], p2_psl[m][:])
                else:
                    nc.scalar.copy(psb[:], p2_psl[m][:])
                p2_sbl.append(psb)
            nk2 = bsb.tile([P, NCHUNK], F32)
            scrap0 = bsb.tile([P, P], F32)
            scrap1 = bsb.tile([P, P], F32)
            scraps = [scrap0, scrap1]
            for m in range(NCHUNK):
                kg = bwork.tile([P, P], F32, space=PSUM, name=f"kgps{m}", tag="bw", bufs=4)
                for k in range(NCHUNK):
                    nc.tensor.matmul(
                        kg[:],
                        wqk_r[:, k, ds(C + m * P, P)],
                        p2_sbl[k][:, ts(m, P)],
                        start=(k == 0),
                        stop=(k == 1),
                    )
                nc.vector.scalar_tensor_tensor(
                    out=scraps[m][:],
                    in0=kg[:],
                    scalar=1.0,
                    in1=identity_f[:],
                    op0=ALU.mult,
                    op1=ALU.mult,
                    accum_out=nk2[:, m : m + 1],
                )
            lnk = bsb.tile([P, NCHUNK], F32)
            nc.scalar.activation(lnk[:], nk2[:], ACTF.Ln)
            rk = bsb.tile([P, NCHUNK], F32)
            nc.scalar.activation(rk[:], lnk[:], ACTF.Exp, scale=-0.5)

            # A is block-diagonal at chunk level too (heads never span the
            # 128-chunks), so the softmax tail runs on the diagonal 128x128
            # blocks only, and T1[m] = a[m]^T Wp[m] is a single matmul with
            # no cross-chunk dependency.
            t1_sbl = []
            for m in range(NCHUNK):
                dg = ds(m * P, P)
                tt = bsb.tile([P, P], F32, name=f"t{m}", tag="t", bufs=2)
                nc.vector.tensor_mul(tt[:], g_psl[m][:, dg], csbc_sb[:, dg])
                e = bsb.tile([P, P], F32, name=f"e{m}", tag="e", bufs=2)
                nc.scalar.activation(e[:], tt[:], ACTF.Exp, scale=rk[:, m : m + 1])
                em = bsb.tile([P, P], F32, name=f"em{m}", tag="em", bufs=2)
                den = bsb.tile([P, 1], F32, name=f"den{m}", tag="den", bufs=2)
                nc.vector.scalar_tensor_tensor(
                    out=em[:],
                    in0=e[:],
                    scalar=1.0,
                    in1=bdmask[:, m, dg],
                    op0=ALU.mult,
                    op1=ALU.mult,
                    accum_out=den[:],
                )
                rden = bsb.tile([P, 1], F32, name=f"rden{m}", tag="rden", bufs=2)
                nc.vector.reciprocal(rden[:], den[:])
                a_m = bsb.tile([P, P], F32R, name=f"a{m}", tag="a", bufs=2)
                nc.vector.tensor_scalar_mul(a_m[:], em[:], rden[:])
                t1p = bwork.tile([P, C], F32, space=PSUM, name=f"t1ps{m}", tag="bw", bufs=4)
                nc.tensor.matmul(
                    t1p[:], a_m[:], wp_r[:, m, :], start=True, stop=True
                )
                t1s = bsb.tile([P, C], F32R, name=f"t1sb{m}", tag="t1sb", bufs=2)
                if m == 0:
                    nc.vector.tensor_copy(t1s[:], t1p[:])
                else:
                    nc.scalar.copy(t1s[:], t1p[:])
                t1_sbl.append(t1s)

            # Wbig = Wv @ T1  (lhsT = Wv^T chunks)
            for m in range(NCHUNK):
                wbp = bwork.tile([P, C], F32, space=PSUM, name=f"wbps{m}", tag="bw", bufs=4)
                for k in range(NCHUNK):
                    nc.tensor.matmul(
                        wbp[:],
                        wvT[:, k, ts(m, P)],
                        t1_sbl[k][:],
                        start=(k == 0),
                        stop=(k == 1),
                    )
                if m == 0:
                    nc.vector.tensor_copy(wbig_l[m][:], wbp[:])
                else:
                    nc.scalar.copy(wbig_l[m][:], wbp[:])

        s_stack.close()  # free the S PSUM banks for the pass-2 pools

        # ------- pass 2: per group, transpose X tiles then out = X Wbig + bp -------
        bias_v = bias_bc[:].rearrange("p (h c) -> p h c", h=2)
        with tc.tile_pool(name="ops", bufs=4, space=PSUM) as ops, tc.tile_pool(
            name="tpp", bufs=3, space=PSUM
        ) as tpp, tc.tile_pool(name="outb", bufs=4) as outb:

            def emit_transposes(gi):
                # 4 tiles share one PSUM tp tile; a single strided eviction
                # writes all 8 chunk-blocks (one DVE/ACT op per quad)
                gsz = GROUPS[gi]
                for q0 in range(0, gsz, 4):
                    nq = min(4, gsz - q0)
                    tp = tpp.tile([P, 4 * 2 * P], BF16, space=PSUM, tag="tp2")
                    for j in range(nq):
                        t = starts[gi] + q0 + j
                        g, a = divmod(t, DMA_TILES)
                        for k in range(NCHUNK):
                            nc.tensor.transpose(
                                tp[:, ds((j * NCHUNK + k) * P, P)],
                                xg[g][:, a, ts(k, P)],
                                identity[:],
                            )
                    tp_v = tp[:, 0 : nq * NCHUNK * P].rearrange(
                        "p (j k c) -> p k j c", k=NCHUNK, c=P
                    )
                    dst = xTg[gi][:, :, ds(q0 * P, nq * P)].rearrange(
                        "p k (j c) -> p k j c", c=P
                    )
                    if (starts[gi] + q0) % 8 < 4:
                        nc.vector.tensor_copy(dst, tp_v)
                    else:
                        nc.scalar.copy(dst, tp_v)

            pair_idx = 0
            emitted = 0
            for gi, gsz in enumerate(GROUPS):
                t0 = starts[gi]
                # own group's transposes first, deeper lookahead after the
                # out matmuls so the first output DMA isn't delayed
                while emitted <= gi:
                    emit_transposes(emitted)
                    emitted += 1
                ob = outb.tile([P, gsz, C], F32, tag="ob")
                for a2 in range(gsz // 2):
                    o_ps = ops.tile([P, 2 * C], F32, space=PSUM, tag="o")
                    even = pair_idx % 2 == 0
                    for h2 in range(2):
                        j = a2 * 2 + h2
                        for k in range(NCHUNK):
                            nc.tensor.matmul(
                                o_ps[:, ts(h2, C)],
                                xTg[gi][:, k, ts(j, P)],
                                wbig_l[k][:],
                                start=(k == 0),
                                stop=(even and k == 1),
                            )
                        if not even:
                            nc.tensor.matmul(
                                o_ps[:, ts(h2, C)],
                                ones_row_r[:],
                                bp2_r[:, 0:C],
                                start=False,
                                stop=True,
                            )
                    o_v = o_ps[:].rearrange("p (h c) -> p h c", h=2)
                    if even:
                        nc.vector.tensor_add(ob[:, ds(a2 * 2, 2), :], o_v, bias_v)
                    else:
                        nc.scalar.copy(ob[:, ds(a2 * 2, 2), :], o_v)
                    pair_idx += 1
                while emitted <= min(gi + 3, len(GROUPS) - 1):
                    emit_transposes(emitted)
                    emitted += 1
                nc.sync.dma_start(
                    out_dram[ds(t0 * P, gsz * P), :].rearrange(
                        "(a p) c -> p a c", p=P
                    ),
                    ob[:],
                )

    return nc


_NC_CACHE = None


def _get_nc():
    global _NC_CACHE
    if _NC_CACHE is None:
        nc = bacc.Bacc(
            "TRN2",
            target_bir_lowering=False,
            debug=False,
            enable_asserts=False,
            num_devices=N_CORES,
        )
        _build_kernel(nc)
        nc.compile()
        nc.m = get_hw_module(nc.m)
        _NC_CACHE = nc
    return _NC_CACHE


def _make_in_maps(x_in, Wq, Wk, Wv, rescale, Wp, bp):
    x_in = np.ascontiguousarray(np.asarray(x_in, dtype=np.float32))
    maps = []
    for core in range(N_CORES):
        maps.append(
            {
                "x_in": x_in[core].reshape(N, C),
                "Wq": np.asarray(Wq, np.float32),
                "Wk": np.asarray(Wk, np.float32),
                "Wv": np.asarray(Wv, np.float32),
                "rescale": np.asarray(rescale, np.float32),
                "Wp": np.asarray(Wp, np.float32),
                "bp": np.asarray(bp, np.float32),
            }
        )
    return maps


def run_on_hw(inputs: dict, trace: bool = False, tmpdir: str | None = None):
    """Returns (full_output [8,128,128,256] f32, BassKernelResults)."""
    nc = _get_nc()
    in_maps = _make_in_maps(**inputs)
    res = bass_utils.run_bass_kernel_spmd(
        nc, in_maps, core_ids=list(range(N_CORES)), trace=trace, tmpdir=tmpdir
    )
    out = np.stack([res.results[c]["out"].reshape(H, W, C) for c in range(N_CORES)])
    return out.astype(np.float32), res


def kernel(x_in, Wq, Wk, Wv, rescale, Wp, bp) -> np.ndarray:
    out, _ = run_on_hw(
        dict(x_in=x_in, Wq=Wq, Wk=Wk, Wv=Wv, rescale=rescale, Wp=Wp, bp=bp)
    )
    return out
